# revision 23
# baseline (speedup 1.0000x reference)
"""Trainium2 Bass kernel for nn_BinaryTokenClassificationModel (segment_reduce).

Math: the reference pools token embeddings into word embeddings (mean over
contiguous runs of equal word ids), then computes
    logits[b,s,t] = src_pooled[b,s] @ w_src + tgt_pooled[b,t] @ w_tgt + b.
Because the classifier is linear, pooling and projection commute:
    u[t] = tok_h[t] @ w_blk(t)                     (per-token scalar)
    logits[s,t] = sum_t atw_src[t,s] u[t] + sum_t' atw_tgt[t',t] u[t'] + b
where atw is the 1/count-weighted segment membership matrix; the [S, T]
PSUM tile realizes segment-reduce + outer-sum in one matmul per chunk
(the scalar u rides a stride-0 broadcast matmul operand).
Data-parallel over batch: core i handles batch row i. No collectives.

Hardware findings this kernel is shaped around (HW traces, TRN2):
  - GpSimd is poison: pool-library load + drains cost ~12us.  Any ACT
    compute op inserts a 1.3us ACT_TABLE_LOAD ahead of the scalar
    engine's DMA issues.  Neither engine runs compute here.
  - DMA is transfer-count bound, not byte bound: each transfer on a queue
    costs ~1-2.5us (descriptor/semaphore round trip) + bytes at
    ~130-250 B/ns per queue, with ~1us run-to-run jitter.  Hence: chunks
    are packed host-side as [tok | membership] so each chunk is ONE
    contiguous transfer, 2 chunk transfers per HWDGE queue, and W is NOT
    replicated into the stream (a 393KB pre-broadcast constant or a
    stride-0 broadcast DMA both lose to a 3KB row + PE broadcast).
  - The W row is baked into the NEFF as an inline constant (cache keyed
    on W/b bytes; rebuilt if they change), broadcast across partitions by
    two k=1 matmuls per half, and the DVE reduces read the resulting
    multi-bank fp32 PSUM tiles directly as in1 -- no PSUM->SBUF copy.
  - tok_h is cast to bf16 on the host (halves DMA bytes; rel err ~2.6e-3
    vs the 2e-2 gate) and u accumulates straight to bf16 (internal
    accumulation is fp32; avoids a cast op before the matmul).
  - Fixed NEFF scaffolding (~6us excluded preamble, ~8us counted
    teardown: drains + a ~2.1us quiesce gap + a ~4.3us staggered
    all-engine exit barrier) dominates what remains; metric ~= last
    reduce end + ~5.3us.

Engine mapping:
  DVE : 4 fused multiply-reduce ops (u per 128-token chunk), final
        bias-add of the PSUM tile (bias folded in as an immediate)
  PE  : W broadcast (k=1 matmuls), one [128,*]x[128,128] matmul per
        chunk into the [S,T] PSUM tile
  SP/ACT : DMA issue only (sync: c0, c2, out lo; scalar: wrow, c1, c3,
        out hi)
"""

import functools

import numpy as np
import ml_dtypes

import concourse.bacc as bacc
import concourse.mybir as mybir
from concourse.bass_utils import run_bass_kernel_spmd
from concourse.tile import TileContext

# Problem geometry (hardcoded per spec)
B = 8
L_SRC = 256
L_TGT = 256
L = L_SRC + L_TGT  # 512
H = 768
P = 128            # SBUF partitions / tokens per chunk
NCHUNK = L // P    # 4
N_SRC_CHUNKS = L_SRC // P  # 2
N_CORES = 8
F32 = mybir.dt.float32
BF16 = mybir.dt.bfloat16
NPBF16 = ml_dtypes.bfloat16


# ---------------------------------------------------------------------------
# Host-side segment bookkeeping (exact mirror of reference._pool_words)
# ---------------------------------------------------------------------------

def _segments(combined_wid, attention_mask, n_words):
    """Per-token dense run ids exactly as the reference computes them."""
    valid = (attention_mask > 0) & (combined_wid >= 0)  # [B, L]
    prev_wid = np.concatenate(
        [np.full((combined_wid.shape[0], 1), -2, dtype=combined_wid.dtype),
         combined_wid[:, :-1]], axis=1)
    prev_valid = np.concatenate(
        [np.zeros((valid.shape[0], 1), dtype=bool), valid[:, :-1]], axis=1)
    new_run = valid & ((combined_wid != prev_wid) | (~prev_valid))
    run_id = np.cumsum(new_run.astype(np.int64), axis=1) - 1  # [B, L]
    seg = np.where(valid, run_id, n_words)  # n_words = dummy slot
    return seg, valid


def _seg_weights(seg, valid, n_words):
    """1/max(count,1) weight for each token's segment (0 for invalid)."""
    Bv, Lv = seg.shape
    wgt = np.zeros((Bv, Lv), dtype=np.float32)
    for b in range(Bv):
        counts = np.bincount(seg[b][valid[b]], minlength=Lv + 1).astype(np.float32)
        inv = 1.0 / np.maximum(counts, 1.0)
        wgt[b] = np.where(valid[b] & (seg[b] < n_words), inv[np.minimum(seg[b], Lv)], 0.0)
    return wgt


# ---------------------------------------------------------------------------
# Device kernel
# ---------------------------------------------------------------------------

def _emit(nc, tc, S, T, block_ok, wb_np, b_val):
    """block_ok fast path: src tokens only map to word rows [0,S), tgt
    tokens only to [S,S+T) -> each chunk's membership is [128, P] and each
    chunk does ONE reduce.  General path: membership is [128, S+T] and each
    chunk reduces against both weight halves.

    wb_np ([128, 2H] bf16, W pre-broadcast) and b_val (python float bias)
    are baked into the NEFF: wb as an inline DRAM constant (one fast
    striped DMA, no broadcast machinery on device), the bias as an
    immediate on the final DVE add."""
    NW = S + T
    AW = P if block_ok else NW
    CW = H + AW  # packed chunk width: tok columns then membership columns
    # one blob row per partition: [c0 | c1 | c2 | c3].  DMA bandwidth is
    # the binding constraint (~215 B/ns aggregate), so W is NOT replicated
    # into the stream -- a 3KB wrow DMA feeds a k=1 broadcast matmul whose
    # PSUM output the DVE reduces read directly (multi-bank fp32 PSUM APs
    # work as AMR in1).
    BW = NCHUNK * CW
    blob = nc.declare_dram_parameter("blob", [P, BW], BF16, isOutput=False)
    wrow = nc.inline_tensor(wb_np[0:1, :], name="wrow")
    out = nc.declare_dram_parameter("out", [S, T], F32, isOutput=True)
    o_chunk = [c * CW for c in range(NCHUNK)]

    with (
        tc.tile_pool(name="const", bufs=1) as cpool,
        tc.tile_pool(name="prods", bufs=2) as ppool,
        tc.tile_pool(name="psum", bufs=1, space="PSUM") as pspool,
    ):
        blob_sb = cpool.tile([P, BW], BF16)
        # scalar queue: wrow (tiny, gates the broadcast), c1, c3
        # sync queue:   c0, c2 (+ the output halves later)
        wrow_sb = cpool.tile([1, 2 * H], BF16)
        nc.scalar.dma_start(out=wrow_sb[:], in_=wrow[:], single_packet=True)
        for c, eng in ((2, nc.sync), (3, nc.scalar), (0, nc.sync),
                       (1, nc.scalar)):
            eng.dma_start(out=blob_sb[:, o_chunk[c]:o_chunk[c] + CW],
                          in_=blob[:, o_chunk[c]:o_chunk[c] + CW])

        # broadcast W across partitions on the (idle) PE: wb[p, j] = w[j]
        ones_bf = cpool.tile([1, P], BF16)
        nc.vector.memset(ones_bf[:], 1.0)
        wb_ps = [pspool.tile([P, H], F32, name=f"wb{h}") for h in range(2)]
        for h in (1, 0):
            for j0, j1 in ((0, 512), (512, H)):
                nc.tensor.matmul(wb_ps[h][:, j0:j1], ones_bf[:],
                                 wrow_sb[0:1, h * H + j0:h * H + j1],
                                 start=True, stop=True, skip_group_check=True)

        n_u = NCHUNK if block_ok else 2 * NCHUNK
        u_bf = cpool.tile([P, n_u], BF16)
        psum_out = pspool.tile([S, T], F32)

        wb_half = [wb_ps[0][:], wb_ps[1][:]]
        out_sb = cpool.tile([S, T], F32)
        hS = S // 2
        # tgt chunks first: their matmuls carry u as the STATIONARY operand
        # (LDWEIGHTS gated on u), so keeping a src chunk last lets the tail
        # LDWEIGHTS (atw) hoist ahead of the final reduce.  The last src
        # matmul is split by psum rows so each output half can be
        # bias-added and stored the moment its half finishes.
        order = [2, 3, 0, 1] if block_ok else list(range(NCHUNK))
        for oi, c in enumerate(order):
            tok_c = blob_sb[:, o_chunk[c]:o_chunk[c] + H]
            atw_c = blob_sb[:, o_chunk[c] + H:o_chunk[c] + CW]

            if block_ok:
                is_src = c < N_SRC_CHUNKS
                jobs = [(0 if is_src else 1, is_src,
                         atw_c[:, 0:(S if is_src else T)], oi)]
            else:
                jobs = [(0, True, atw_c[:, 0:S], 2 * oi),
                        (1, False, atw_c[:, S:NW], 2 * oi + 1)]

            for half, is_src, atw_ap, ui in jobs:
                prod = ppool.tile([P, H], BF16, name=f"prod{ui % 2}")
                with nc.allow_low_precision(
                        reason="u feeds a bf16 matmul operand anyway"):
                    nc.vector.affine_mul_reduce(
                        out=prod[:], accum_out=u_bf[:, ui:ui + 1], in0=tok_c,
                        in1=wb_half[half], scale=1.0, bias=0.0)
                ub = u_bf[:, ui:ui + 1]
                first = ui == 0
                last = ui == n_u - 1
                if last and is_src and block_ok:
                    nc.tensor.matmul(psum_out[:], atw_ap, ub.broadcast_to([P, T]),
                                     start=False, stop=True, skip_group_check=True)
                    nc.vector.tensor_scalar_add(out_sb[:], psum_out[:],
                                                float(b_val))
                    nc.sync.dma_start(out=out[0:hS, :], in_=out_sb[0:hS, :])
                    nc.scalar.dma_start(out=out[hS:S, :], in_=out_sb[hS:S, :])
                elif is_src:
                    nc.tensor.matmul(psum_out[:], atw_ap, ub.broadcast_to([P, T]),
                                     start=first, stop=last, skip_group_check=True)
                else:
                    nc.tensor.matmul(psum_out[:], ub.broadcast_to([P, S]), atw_ap,
                                     start=first, stop=last, skip_group_check=True)

        if not block_ok:
            nc.vector.tensor_scalar_add(out_sb[:], psum_out[:], float(b_val))
            nc.sync.dma_start(out=out[0:hS, :], in_=out_sb[0:hS, :])
            nc.scalar.dma_start(out=out[hS:S, :], in_=out_sb[hS:S, :])


@functools.lru_cache(maxsize=4)
def _build(S, T, block_ok, wb_bytes, b_val):
    wb_np = np.frombuffer(wb_bytes, dtype=NPBF16).reshape(P, 2 * H)
    nc = bacc.Bacc("TRN2", debug=False, num_devices=N_CORES)
    with TileContext(nc) as tc:
        _emit(nc, tc, S, T, block_ok, wb_np, b_val)
    nc.compile()
    return nc


# ---------------------------------------------------------------------------
# Host wrapper
# ---------------------------------------------------------------------------

def _prep(inputs):
    tok_h = np.ascontiguousarray(np.asarray(inputs["tok_h"], dtype=np.float32))
    mask = np.asarray(inputs["attention_mask"])
    swid = np.asarray(inputs["source_word_ids"])
    twid = np.asarray(inputs["target_word_ids"])
    W = np.asarray(inputs["W"], dtype=np.float32)
    b = np.asarray(inputs["b"], dtype=np.float32)
    S = int(np.asarray(inputs["S"]))
    T = int(np.asarray(inputs["T"]))

    Bv, Lv, Hv = tok_h.shape
    assert (Bv, Lv, Hv) == (B, L, H), f"unexpected tok_h shape {tok_h.shape}"
    assert swid.shape == (B, L_SRC) and twid.shape == (B, L_TGT)
    assert S <= P and T <= P

    NW = S + T
    combined = np.concatenate([swid, twid], axis=1).astype(np.int64)
    seg, valid = _segments(combined, mask, NW)
    wgt = _seg_weights(seg, valid, NW)

    src_tok_seg = seg[:, :L_SRC][valid[:, :L_SRC]]
    tgt_tok_seg = seg[:, L_SRC:][valid[:, L_SRC:]]
    block_ok = bool(
        (src_tok_seg < S).all()
        and (tgt_tok_seg >= S).all() and (tgt_tok_seg < NW).all()
    )

    wrow_bf = np.concatenate([W[:H, 0], W[H:2 * H, 0]]).reshape(1, 2 * H).astype(NPBF16)
    wb_np = np.ascontiguousarray(np.broadcast_to(wrow_bf, (P, 2 * H)))
    b_val = float(b.reshape(-1)[0])

    AW = P if block_ok else NW
    CW = H + AW
    tidx = np.arange(L)
    tok_bf = tok_h.astype(NPBF16)
    in_maps = []
    for bi in range(B):
        atw_f = np.zeros((L, AW), dtype=np.float32)
        segb = seg[bi]
        ok = valid[bi] & (segb < NW)
        if block_ok:
            col = np.where(tidx < L_SRC, segb, segb - S)
        else:
            col = segb
        atw_f[tidx[ok], col[ok]] = wgt[bi][ok]
        packed = np.empty((NCHUNK, P, CW), dtype=NPBF16)
        packed[:, :, 0:H] = tok_bf[bi].reshape(NCHUNK, P, H)
        packed[:, :, H:CW] = atw_f.astype(NPBF16).reshape(NCHUNK, P, AW)
        # blob row layout: [c0 | c1 | c2 | c3]
        blob = np.concatenate([packed[0], packed[1], packed[2], packed[3]],
                              axis=1)
        in_maps.append({"blob": np.ascontiguousarray(blob)})
    return S, T, block_ok, wb_np, b_val, in_maps


def kernel(**inputs):
    S, T, block_ok, wb_np, b_val, in_maps = _prep(inputs)
    nc = _build(S, T, block_ok, wb_np.tobytes(), b_val)
    res = run_bass_kernel_spmd(nc, in_maps, core_ids=list(range(N_CORES)))
    return np.stack([res.results[i]["out"] for i in range(B)], axis=0)
